# revision 8
# baseline (speedup 1.0000x reference)
"""Trainium2 Bass kernel for nn_BaseNAM (per-feature tiny MLPs / NAM).

Strategy
--------
Data-parallel over batch: 8 cores x 1024 rows each, no collectives.

Math trick for missing-value handling: with x' = x * (1 - miss),
    feat_masked = (1-m) * MLP(x) + m * emb
                = MLP(x') + m * (emb - MLP(0))
since for m in {0,1}: m=0 -> MLP(x); m=1 -> x'=0 -> MLP(0), and the
correction m*(emb - c) with c = MLP_f(0) (a host-precomputed constant)
fixes it up exactly.  This makes masking + missing-embedding injection
pure PSUM-accumulated matmuls -- no elementwise mask work on DVE/ACT.

Per-core pipeline (feature-major layout [feature-dims, batch] for the MLP):
  - PE-transpose tabular chunks -> xT/missT [96, 1024]; x'T = xT*(1-missT)
  - L0/L1/L2: per 2-feature pack, block-diagonal matmuls (K<=128), ACT/DVE
    evacuation with fused bias+relu -> h2 "quad" tiles [128=(4f x 32), 1024]
  - logits: stacked-W3 selector matmuls accumulate sum_f W3_f h2_f directly
    in PSUM, plus matmul corrections for miss-emb and cat features
  - L3 batch-major: psum[b, (f,o)] = h2quad_slice.T @ blockdiag(W3) (+ miss
    corrections, + cat features) -> features output is written batch-major,
    DMA'd contiguously.
"""

import sys

import numpy as np

sys.path.insert(0, "/opt/trn_rl_repo")

F_REAL = 64
F_CAT = 32
OUT = 32
B = 8192
NCORES = 8
BT = B // NCORES  # 1024 rows per core

_BUILT = None  # cached (nc, const_names)


def _wn(v, g):
    v = np.asarray(v, np.float32)
    g = np.asarray(g, np.float32)
    n = np.sqrt((v * v).sum(-1, keepdims=True)).astype(np.float32)
    return (g[..., None] * v / n).astype(np.float32)


def _prep_consts(inp):
    f32 = np.float32
    b0 = np.asarray(inp["b0"], f32)
    b1 = np.asarray(inp["b1"], f32)
    b2 = np.asarray(inp["b2"], f32)
    w0 = _wn(inp["v0"], inp["g0"])[:, :, 0]  # [64, 64] (f, j)
    W1 = _wn(inp["v1"], inp["g1"])           # [64, 64, 64] (f, o, i)
    W2 = _wn(inp["v2"], inp["g2"])           # [64, 32, 64]
    W3 = _wn(inp["v3"], inp["g3"])           # [64, 32, 32]
    CL = np.asarray(inp["cat_linear"], f32)  # [32, 32]
    ME = np.asarray(inp["missing_emb"], f32) # [24, 32] (cat 0:8, real 8:24)
    bias = np.asarray(inp["bias"], f32)      # [1, 32]

    # c_f = MLP_f(0)
    h0 = np.maximum(b0, 0.0)
    h1 = np.maximum(np.einsum("foi,fi->fo", W1, h0).astype(f32) + b1, 0.0)
    h2 = np.maximum(np.einsum("foi,fi->fo", W2, h1).astype(f32) + b2, 0.0)
    cf = np.einsum("foi,fi->fo", W3, h2).astype(f32)  # [64, 32]
    embp = (-cf).astype(f32)
    embp[:16] += ME[8:24]  # emb'_f = emb_f - c_f (f<16), -c_f otherwise

    C = {}
    # L0 selector weights: [k=64, pack=32, j=128]
    l0w = np.zeros((64, 32, 128), f32)
    for p in range(32):
        for s in range(2):
            l0w[2 * p + s, p, s * 64:(s + 1) * 64] = w0[2 * p + s]
    C["l0w"] = l0w
    # L1 block-diag: [k=128, pack, j=128]
    l1w = np.zeros((128, 32, 128), f32)
    for p in range(32):
        l1w[0:64, p, 0:64] = W1[2 * p].T
        l1w[64:128, p, 64:128] = W1[2 * p + 1].T
    C["l1w"] = l1w
    # L2 block-diag: [k=128, pack, j=64]
    l2w = np.zeros((128, 32, 64), f32)
    for p in range(32):
        l2w[0:64, p, 0:32] = W2[2 * p].T
        l2w[64:128, p, 32:64] = W2[2 * p + 1].T
    C["l2w"] = l2w
    # L3 block-diag per quad: [k=128, quad=16, j=128]
    l3w = np.zeros((128, 16, 128), f32)
    for t in range(16):
        for q in range(4):
            l3w[q * 32:(q + 1) * 32, t, q * 32:(q + 1) * 32] = W3[4 * t + q].T
    C["l3w"] = l3w
    # stacked W3 for logits: [k=128, quad, o=32]
    w3s = np.zeros((128, 16, 32), f32)
    for t in range(16):
        for q in range(4):
            w3s[q * 32:(q + 1) * 32, t, :] = W3[4 * t + q].T
    C["w3s"] = w3s
    # biases, per-partition columns
    b0c = np.zeros((128, 32), f32)
    b1c = np.zeros((128, 32), f32)
    for p in range(32):
        b0c[0:64, p] = b0[2 * p]
        b0c[64:128, p] = b0[2 * p + 1]
        b1c[0:64, p] = b1[2 * p]
        b1c[64:128, p] = b1[2 * p + 1]
    C["b0c"] = b0c
    C["b1c"] = b1c
    b2q = np.zeros((128, 16), f32)
    for t in range(16):
        for q in range(4):
            b2q[q * 32:(q + 1) * 32, t] = b2[4 * t + q]
    C["b2q"] = b2q
    # miss-emb correction rhs, real features: [64, 2048]
    embR = np.zeros((64, 2048), f32)
    for f in range(64):
        embR[f, f * 32:(f + 1) * 32] = embp[f]
    C["embR"] = embR
    # cat-feature constants are padded so their data sits at partitions 64+,
    # matching the base partition of the xpT[64:96] / missT[64:72] operands
    # (matmul requires lhsT and rhs to share a base partition).
    # cat linear block-diag rhs: [96, 1024], rows 64:96
    catrhs = np.zeros((96, 1024), f32)
    for fc in range(32):
        catrhs[64 + fc, fc * 32:(fc + 1) * 32] = CL[fc]
    C["catrhs"] = catrhs
    # cat miss-emb rhs: [72, 256], rows 64:72
    embC8 = np.zeros((72, 256), f32)
    for fc in range(8):
        embC8[64 + fc, fc * 32:(fc + 1) * 32] = ME[fc]
    C["embC8"] = embC8
    # logits correction weights
    C["embRs"] = embp                      # [64, 32]
    embC8s = np.zeros((72, 32), f32)
    embC8s[64:72] = ME[:8]
    C["embC8s"] = embC8s
    catlin = np.zeros((96, 32), f32)
    catlin[64:96] = CL
    C["catlin"] = catlin
    C["biasB"] = np.broadcast_to(bias, (128, 32)).copy()  # [128, 32]
    return C


_CONST_SPECS = [
    ("l0w", [64, 32, 128]),
    ("l1w", [128, 32, 128]),
    ("l2w", [128, 32, 64]),
    ("l3w", [128, 16, 128]),
    ("w3s", [128, 16, 32]),
    ("b0c", [128, 32]),
    ("b1c", [128, 32]),
    ("b2q", [128, 16]),
    ("embR", [64, 2048]),
    ("catrhs", [96, 1024]),
    ("embC8", [72, 256]),
    ("embRs", [64, 32]),
    ("embC8s", [72, 32]),
    ("catlin", [96, 32]),
    ("biasB", [128, 32]),
]


def _build_nc():
    import concourse.mybir as mybir
    import concourse.tile as tile
    from concourse import bacc
    from concourse.masks import make_identity
    from contextlib import ExitStack

    f32 = mybir.dt.float32
    Relu = mybir.ActivationFunctionType.Relu
    mult = mybir.AluOpType.mult
    add = mybir.AluOpType.add
    amax = mybir.AluOpType.max

    nc = bacc.Bacc(None, target_bir_lowering=False)
    tab_d = nc.declare_dram_parameter("tab", [BT, 2, 96], f32, isOutput=False)
    cdram = {}
    for name, shape in _CONST_SPECS:
        cdram[name] = nc.declare_dram_parameter(name, shape, f32, isOutput=False)
    feat_d = nc.declare_dram_parameter("features_out", [BT, 96, 32], f32, isOutput=True)
    log_d = nc.declare_dram_parameter("logits_out", [BT, 32], f32, isOutput=True)

    with ExitStack() as ctx:
        tc = ctx.enter_context(tile.TileContext(nc))
        consts = ctx.enter_context(tc.tile_pool(name="consts", bufs=1))
        h2pool = ctx.enter_context(tc.tile_pool(name="h2", bufs=16))

        ct = {}
        for name, shape in _CONST_SPECS:
            t = consts.tile(shape, f32, tag=name)
            nc.sync.dma_start(out=t[...], in_=cdram[name][...])
            ct[name] = t
        ident = consts.tile([128, 128], f32, tag="ident")
        make_identity(nc, ident[...])

        xpT = consts.tile([96, 1024], f32, tag="xpT")    # x * (1-m), feature-major
        missT = consts.tile([96, 1024], f32, tag="missT")

        # ---- Phase A0: transpose tabular to feature-major ----
        with tc.tile_pool(name="tp_psum", bufs=4, space="PSUM") as tpp, \
             tc.tile_pool(name="tabw", bufs=3) as tabw:
            for c in range(8):
                tabc = tabw.tile([128, 2, 96], f32, tag="tabc")
                nc.sync.dma_start(
                    out=tabc[...],
                    in_=tab_d[c * 128:(c + 1) * 128, :, :],
                )
                pt = tpp.tile([96, 128], f32, tag="pt")
                nc.tensor.transpose(pt[...], tabc[:, 0, :], ident[...])
                nc.scalar.copy(out=xpT[:, c * 128:(c + 1) * 128], in_=pt[...])
                pm = tpp.tile([96, 128], f32, tag="pt")
                nc.tensor.transpose(pm[...], tabc[:, 1, :], ident[...])
                nc.scalar.copy(out=missT[:, c * 128:(c + 1) * 128], in_=pm[...])
            m1T = tabw.tile([96, 1024], f32, tag="m1T")
            # m1T = 1 - missT
            nc.vector.tensor_scalar(
                out=m1T[...], in0=missT[...], scalar1=-1.0, scalar2=1.0,
                op0=mult, op1=add,
            )
            # xpT *= m1T  (in-place: xpT currently holds raw x^T)
            nc.vector.tensor_mul(xpT[...], xpT[...], m1T[...])

        # ---- Phase A: per-pack MLP chain L0 -> L1 -> L2 ----
        h2qs = []
        with tc.tile_pool(name="mm_psum", bufs=3, space="PSUM") as mmp, \
             tc.tile_pool(name="hwork", bufs=3) as hwork:
            for t in range(16):
                ps2 = mmp.tile([128, 1024], f32, tag="ps")
                h2q = h2pool.tile([128, 1024], f32, tag="h2q")
                for s2 in range(2):
                    p = 2 * t + s2
                    ps0 = mmp.tile([128, 1024], f32, tag="ps")
                    for n in range(2):
                        nc.tensor.matmul(
                            ps0[:, n * 512:(n + 1) * 512],
                            ct["l0w"][:, p, :],
                            xpT[0:64, n * 512:(n + 1) * 512],
                            start=True, stop=True,
                        )
                    h0 = hwork.tile([128, 1024], f32, tag="h0")
                    nc.scalar.activation(h0[...], ps0[...], Relu,
                                         bias=ct["b0c"][:, p:p + 1])
                    ps1 = mmp.tile([128, 1024], f32, tag="ps")
                    for n in range(2):
                        nc.tensor.matmul(
                            ps1[:, n * 512:(n + 1) * 512],
                            ct["l1w"][:, p, :],
                            h0[:, n * 512:(n + 1) * 512],
                            start=True, stop=True,
                        )
                    h1 = hwork.tile([128, 1024], f32, tag="h1")
                    # relu(x + b) on DVE: max(x + b, 0)
                    nc.vector.tensor_scalar(
                        out=h1[...], in0=ps1[...],
                        scalar1=ct["b1c"][:, p:p + 1], scalar2=0.0,
                        op0=add, op1=amax,
                    )
                    for n in range(2):
                        nc.tensor.matmul(
                            ps2[s2 * 64:(s2 + 1) * 64, n * 512:(n + 1) * 512],
                            ct["l2w"][:, p, :],
                            h1[:, n * 512:(n + 1) * 512],
                            start=True, stop=True,
                            tile_position=(0, 64 * s2),
                        )
                nc.scalar.activation(h2q[...], ps2[...], Relu,
                                     bias=ct["b2q"][:, t:t + 1])
                h2qs.append(h2q)

        # ---- Logits: accumulate sum_f W3_f h2_f (+ corrections) in PSUM ----
        with tc.tile_pool(name="lg_psum", bufs=1, space="PSUM") as lgp, \
             tc.tile_pool(name="lt_psum", bufs=2, space="PSUM") as ltp, \
             tc.tile_pool(name="lwork", bufs=1) as lwork:
            lps = lgp.tile([32, 1024], f32, tag="lps")
            for n in range(2):
                ns = slice(n * 512, (n + 1) * 512)
                for i in range(16):
                    nc.tensor.matmul(lps[:, ns], ct["w3s"][:, i, :],
                                     h2qs[i][:, ns], start=(i == 0), stop=False)
                nc.tensor.matmul(lps[:, ns], ct["embRs"][...],
                                 missT[0:64, ns], start=False, stop=False)
                nc.tensor.matmul(lps[:, ns], ct["embC8s"][64:72, :],
                                 missT[64:72, ns], start=False, stop=False)
                nc.tensor.matmul(lps[:, ns], ct["catlin"][64:96, :],
                                 xpT[64:96, ns], start=False, stop=True)
            lts = lwork.tile([32, 1024], f32, tag="lts")
            nc.scalar.copy(out=lts[...], in_=lps[...])
            lout = lwork.tile([128, 8, 32], f32, tag="lout")
            for c in range(8):
                lt2 = ltp.tile([128, 32], f32, tag="lt2")
                nc.tensor.transpose(lt2[...], lts[:, c * 128:(c + 1) * 128],
                                    ident[0:32, 0:32])
                nc.vector.tensor_add(lout[:, c, :], lt2[...], ct["biasB"][...])
            lo_view = log_d[...].rearrange("(c p) o -> p c o", p=128)
            nc.sync.dma_start(out=lo_view, in_=lout[...])

        # ---- Phase B: batch-major L3 + cat + output ----
        with tc.tile_pool(name="fb_psum", bufs=1, space="PSUM") as fbp, \
             tc.tile_pool(name="cat_psum", bufs=2, space="PSUM") as ctp, \
             tc.tile_pool(name="featw", bufs=2) as featw:
            for s in range(8):
                bs = slice(s * 128, (s + 1) * 128)
                fps = fbp.tile([128, 2048], f32, tag="fps")
                for t in range(16):
                    # one PSUM bank holds 4 quads (512 cols); only the first
                    # matmul touching a bank starts its accumulation group
                    nc.tensor.matmul(
                        fps[:, t * 128:(t + 1) * 128],
                        h2qs[t][:, bs], ct["l3w"][:, t, :],
                        start=(t % 4 == 0), stop=False,
                    )
                for g in range(4):
                    nc.tensor.matmul(
                        fps[:, g * 512:(g + 1) * 512],
                        missT[0:64, bs], ct["embR"][:, g * 512:(g + 1) * 512],
                        start=False, stop=True,
                    )
                cps = ctp.tile([128, 1024], f32, tag="cps")
                nc.tensor.matmul(cps[:, 0:512], xpT[64:96, bs],
                                 ct["catrhs"][64:96, 0:512], start=True, stop=False)
                nc.tensor.matmul(cps[:, 512:1024], xpT[64:96, bs],
                                 ct["catrhs"][64:96, 512:1024], start=True, stop=True)
                nc.tensor.matmul(cps[:, 0:256], missT[64:72, bs],
                                 ct["embC8"][64:72, :], start=False, stop=True)
                fc = featw.tile([128, 3072], f32, tag="featc")
                nc.scalar.copy(out=fc[:, 0:2048], in_=fps[...])
                nc.vector.tensor_copy(fc[:, 2048:3072], cps[...])
                nc.sync.dma_start(
                    out=feat_d[...].rearrange("b f o -> b (f o)")[bs, :],
                    in_=fc[...],
                )

    nc.compile()
    return nc


def _get_nc():
    global _BUILT
    if _BUILT is None:
        _BUILT = _build_nc()
    return _BUILT


def _run(inputs, trace=False):
    from concourse.bass_utils import run_bass_kernel_spmd

    tab = np.asarray(inputs["tabular"], np.float32)
    consts = _prep_consts(inputs)
    nc = _get_nc()
    in_maps = []
    for c in range(NCORES):
        m = {k: v for k, v in consts.items()}
        m["tab"] = np.ascontiguousarray(tab[c * BT:(c + 1) * BT])
        in_maps.append(m)
    bkr = run_bass_kernel_spmd(nc, in_maps, list(range(NCORES)), trace=trace)
    feats = np.concatenate([r["features_out"] for r in bkr.results], axis=0)
    logits = np.concatenate([r["logits_out"] for r in bkr.results], axis=0)
    return logits, feats, bkr


def kernel(**inputs):
    logits, feats, _ = _run(inputs, trace=False)
    return logits, feats


# revision 15
# speedup vs baseline: 1.5997x; 1.5997x over previous
"""Trainium2 Bass kernel for nn_BaseNAM (per-feature tiny MLPs / NAM).

Strategy
--------
Data-parallel over batch: 8 cores x 1024 rows each, no collectives.

Math trick for missing-value handling: with x' = x * (1 - miss),
    feat_masked = (1-m) * MLP(x) + m * emb
                = MLP(x') + m * (emb - MLP(0))
since for m in {0,1}: m=0 -> MLP(x); m=1 -> x'=0 -> MLP(0), and the
correction m*(emb - c) with c = MLP_f(0) (a host-precomputed constant)
fixes it up exactly.  This makes masking + missing-embedding injection
pure PSUM-accumulated matmuls -- no elementwise mask work on DVE/ACT.

Per-core pipeline (feature-major layout [feature-dims, batch] for the MLP):
  - PE-transpose tabular chunks -> xT/missT [96, 1024]; x'T = xT*(1-missT)
  - L0/L1/L2: per 2-feature pack, block-diagonal matmuls (K<=128), ACT/DVE
    evacuation with fused bias+relu -> h2 "quad" tiles [128=(4f x 32), 1024]
  - logits: stacked-W3 selector matmuls accumulate sum_f W3_f h2_f directly
    in PSUM, plus matmul corrections for miss-emb and cat features
  - L3 batch-major: psum[b, (f,o)] = h2quad_slice.T @ blockdiag(W3) (+ miss
    corrections, + cat features) -> features output is written batch-major,
    DMA'd contiguously.
"""

import sys

import numpy as np

sys.path.insert(0, "/opt/trn_rl_repo")

F_REAL = 64
F_CAT = 32
OUT = 32
B = 8192
NCORES = 8
BT = B // NCORES  # 1024 rows per core

_BUILT = None  # cached (nc, const_names)


def _wn(v, g):
    v = np.asarray(v, np.float32)
    g = np.asarray(g, np.float32)
    n = np.sqrt((v * v).sum(-1, keepdims=True)).astype(np.float32)
    return (g[..., None] * v / n).astype(np.float32)


def _prep_consts(inp):
    f32 = np.float32
    b0 = np.asarray(inp["b0"], f32)
    b1 = np.asarray(inp["b1"], f32)
    b2 = np.asarray(inp["b2"], f32)
    w0 = _wn(inp["v0"], inp["g0"])[:, :, 0]  # [64, 64] (f, j)
    W1 = _wn(inp["v1"], inp["g1"])           # [64, 64, 64] (f, o, i)
    W2 = _wn(inp["v2"], inp["g2"])           # [64, 32, 64]
    W3 = _wn(inp["v3"], inp["g3"])           # [64, 32, 32]
    CL = np.asarray(inp["cat_linear"], f32)  # [32, 32]
    ME = np.asarray(inp["missing_emb"], f32) # [24, 32] (cat 0:8, real 8:24)
    bias = np.asarray(inp["bias"], f32)      # [1, 32]

    # c_f = MLP_f(0)
    h0 = np.maximum(b0, 0.0)
    h1 = np.maximum(np.einsum("foi,fi->fo", W1, h0).astype(f32) + b1, 0.0)
    h2 = np.maximum(np.einsum("foi,fi->fo", W2, h1).astype(f32) + b2, 0.0)
    cf = np.einsum("foi,fi->fo", W3, h2).astype(f32)  # [64, 32]
    embp = (-cf).astype(f32)
    embp[:16] += ME[8:24]  # emb'_f = emb_f - c_f (f<16), -c_f otherwise

    C = {}
    # L0 selector weights: [k=64, pack=32, j=128]
    l0w = np.zeros((64, 32, 128), f32)
    for p in range(32):
        for s in range(2):
            l0w[2 * p + s, p, s * 64:(s + 1) * 64] = w0[2 * p + s]
    C["l0w"] = l0w
    # L1 block-diag: [k=128, pack, j=128]
    l1w = np.zeros((128, 32, 128), f32)
    for p in range(32):
        l1w[0:64, p, 0:64] = W1[2 * p].T
        l1w[64:128, p, 64:128] = W1[2 * p + 1].T
    C["l1w"] = l1w
    # L2 block-diag: [k=128, pack, j=64]
    import ml_dtypes
    bf16 = ml_dtypes.bfloat16
    l2w = np.zeros((128, 32, 64), f32)
    for p in range(32):
        l2w[0:64, p, 0:32] = W2[2 * p].T
        l2w[64:128, p, 32:64] = W2[2 * p + 1].T
    C["l2w"] = l2w.astype(bf16)
    # L3 block-diag per quad: [k=128, quad=16, j=128] (bf16: N=128 matmuls
    # run at full rate in bf16 but 4x slower in fp32/f32r)
    l3w = np.zeros((128, 16, 128), f32)
    for t in range(16):
        for q in range(4):
            l3w[q * 32:(q + 1) * 32, t, q * 32:(q + 1) * 32] = W3[4 * t + q].T
    C["l3w"] = l3w.astype(bf16)
    # stacked W3 for logits: [k=128, quad, o=32]
    w3s = np.zeros((128, 16, 32), f32)
    for t in range(16):
        for q in range(4):
            w3s[q * 32:(q + 1) * 32, t, :] = W3[4 * t + q].T
    C["w3s"] = w3s.astype(bf16)
    # biases, per-partition columns
    b0c = np.zeros((128, 32), f32)
    b1c = np.zeros((128, 32), f32)
    for p in range(32):
        b0c[0:64, p] = b0[2 * p]
        b0c[64:128, p] = b0[2 * p + 1]
        b1c[0:64, p] = b1[2 * p]
        b1c[64:128, p] = b1[2 * p + 1]
    C["b0c"] = b0c
    C["b1c"] = b1c
    b2q = np.zeros((128, 16), f32)
    for t in range(16):
        for q in range(4):
            b2q[q * 32:(q + 1) * 32, t] = b2[4 * t + q]
    C["b2q"] = b2q
    # miss-emb correction rhs, real features: [64, 2048]
    embR = np.zeros((64, 2048), f32)
    for f in range(64):
        embR[f, f * 32:(f + 1) * 32] = embp[f]
    C["embR"] = embR
    # cat-feature constants are padded so their data sits at partitions 64+,
    # matching the base partition of the xpT[64:96] / missT[64:72] operands
    # (matmul requires lhsT and rhs to share a base partition).
    # cat linear block-diag rhs: [96, 1024], rows 64:96
    catrhs = np.zeros((96, 1024), f32)
    for fc in range(32):
        catrhs[64 + fc, fc * 32:(fc + 1) * 32] = CL[fc]
    C["catrhs"] = catrhs
    # cat miss-emb rhs: [72, 256], rows 64:72
    embC8 = np.zeros((72, 256), f32)
    for fc in range(8):
        embC8[64 + fc, fc * 32:(fc + 1) * 32] = ME[fc]
    C["embC8"] = embC8
    # logits correction weights
    C["embRs"] = embp                      # [64, 32]
    embC8s = np.zeros((72, 32), f32)
    embC8s[64:72] = ME[:8]
    C["embC8s"] = embC8s
    catlin = np.zeros((96, 32), f32)
    catlin[64:96] = CL
    C["catlin"] = catlin
    C["biasB"] = np.broadcast_to(bias, (128, 32)).copy()  # [128, 32]
    C["ident"] = np.eye(128, dtype=f32)  # PE-transpose identity
    return C


# kind: f32r = matmul-only constant (float32r, full-rate PE at N>=256),
#        bf16 = L3/logits-path constant, f32 = ACT/DVE-read constant
_CONST_SPECS = [
    ("l0w", [64, 32, 128], "f32r"),
    ("l1w", [128, 32, 128], "f32r"),
    ("l2w", [128, 32, 64], "bf16"),
    ("l3w", [128, 16, 128], "bf16"),
    ("w3s", [128, 16, 32], "bf16"),
    ("b0c", [128, 32], "f32"),
    ("b1c", [128, 32], "f32"),
    ("b2q", [128, 16], "f32"),
    ("embR", [64, 2048], "f32r"),
    ("catrhs", [96, 1024], "f32r"),
    ("embC8", [72, 256], "f32r"),
    ("embRs", [64, 32], "f32r"),
    ("embC8s", [72, 32], "f32r"),
    ("catlin", [96, 32], "f32r"),
    ("biasB", [128, 32], "f32"),
    ("ident", [128, 128], "f32r"),
]


def _build_nc():
    import concourse.mybir as mybir
    import concourse.tile as tile
    from concourse import bacc
    from contextlib import ExitStack

    f32 = mybir.dt.float32
    f32r = mybir.dt.float32r
    bf16 = mybir.dt.bfloat16
    _DT = {"f32": f32, "f32r": f32r, "bf16": bf16}
    Relu = mybir.ActivationFunctionType.Relu
    mult = mybir.AluOpType.mult
    add = mybir.AluOpType.add
    amax = mybir.AluOpType.max

    nc = bacc.Bacc(None, target_bir_lowering=False)
    tab_d = nc.declare_dram_parameter("tab", [BT, 2, 96], f32r, isOutput=False)
    cdram = {}
    for name, shape, kind in _CONST_SPECS:
        cdram[name] = nc.declare_dram_parameter(name, shape, _DT[kind], isOutput=False)
    feat_d = nc.declare_dram_parameter("features_out", [BT, 96, 32], f32, isOutput=True)
    log_d = nc.declare_dram_parameter("logits_out", [BT, 32], f32, isOutput=True)

    with ExitStack() as ctx:
        tc = ctx.enter_context(tile.TileContext(nc))
        consts = ctx.enter_context(tc.tile_pool(name="consts", bufs=1))
        h2pool = ctx.enter_context(tc.tile_pool(name="h2", bufs=16))

        ct = {}
        for name, shape, kind in _CONST_SPECS:
            t = consts.tile(shape, _DT[kind], tag=name)
            nc.sync.dma_start(out=t[...], in_=cdram[name][...])
            ct[name] = t

        xpT = consts.tile([96, 1024], f32r, tag="xpT")    # x * (1-m), feature-major
        missT = consts.tile([96, 1024], f32r, tag="missT")

        # ---- Phase A0: transpose tabular to feature-major ----
        with tc.tile_pool(name="tp_psum", bufs=4, space="PSUM") as tpp, \
             tc.tile_pool(name="tabw", bufs=3) as tabw:
            for c in range(8):
                tabc = tabw.tile([128, 2, 96], f32r, tag="tabc")
                nc.sync.dma_start(
                    out=tabc[...],
                    in_=tab_d[c * 128:(c + 1) * 128, :, :],
                )
                pt = tpp.tile([96, 128], f32r, tag="pt")
                nc.tensor.transpose(pt[...], tabc[:, 0, :], ct["ident"][...])
                nc.scalar.copy(out=xpT[:, c * 128:(c + 1) * 128], in_=pt[...])
                pm = tpp.tile([96, 128], f32r, tag="pt")
                nc.tensor.transpose(pm[...], tabc[:, 1, :], ct["ident"][...])
                nc.scalar.copy(out=missT[:, c * 128:(c + 1) * 128], in_=pm[...])
            m1T = tabw.tile([96, 1024], f32, tag="m1T")
            # m1T = 1 - missT
            nc.vector.tensor_scalar(
                out=m1T[...], in0=missT[...], scalar1=-1.0, scalar2=1.0,
                op0=mult, op1=add,
            )
            # xpT *= m1T  (in-place: xpT currently holds raw x^T)
            nc.vector.tensor_mul(xpT[...], xpT[...], m1T[...])

        # ---- Phase A: per-pack MLP chain L0 -> L1 -> L2 ----
        h2qs = []
        with tc.tile_pool(name="mm_psum", bufs=3, space="PSUM") as mmp, \
             tc.tile_pool(name="hwork", bufs=3) as hwork:
            for t in range(16):
                ps2 = mmp.tile([128, 1024], f32, tag="ps")
                h2q = h2pool.tile([128, 1024], bf16, tag="h2q")
                for s2 in range(2):
                    p = 2 * t + s2
                    ps0 = mmp.tile([128, 1024], f32, tag="ps")
                    for n in range(2):
                        nc.tensor.matmul(
                            ps0[:, n * 512:(n + 1) * 512],
                            ct["l0w"][:, p, :],
                            xpT[0:64, n * 512:(n + 1) * 512],
                            start=True, stop=True,
                        )
                    h0 = hwork.tile([128, 1024], f32r, tag="h0")
                    nc.scalar.activation(h0[...], ps0[...], Relu,
                                         bias=ct["b0c"][:, p:p + 1])
                    ps1 = mmp.tile([128, 1024], f32, tag="ps")
                    for n in range(2):
                        nc.tensor.matmul(
                            ps1[:, n * 512:(n + 1) * 512],
                            ct["l1w"][:, p, :],
                            h0[:, n * 512:(n + 1) * 512],
                            start=True, stop=True,
                        )
                    h1 = hwork.tile([128, 1024], bf16, tag="h1")
                    # relu(x + b) on DVE: max(x + b, 0)
                    nc.vector.tensor_scalar(
                        out=h1[...], in0=ps1[...],
                        scalar1=ct["b1c"][:, p:p + 1], scalar2=0.0,
                        op0=add, op1=amax,
                    )
                    for n in range(2):
                        nc.tensor.matmul(
                            ps2[s2 * 64:(s2 + 1) * 64, n * 512:(n + 1) * 512],
                            ct["l2w"][:, p, :],
                            h1[:, n * 512:(n + 1) * 512],
                            start=True, stop=True,
                            tile_position=(0, 64 * s2),
                        )
                nc.scalar.activation(h2q[...], ps2[...], Relu,
                                     bias=ct["b2q"][:, t:t + 1])
                h2qs.append(h2q)

        # ---- Logits: accumulate sum_f W3_f h2_f (+ corrections) in PSUM ----
        with tc.tile_pool(name="lg_psum", bufs=1, space="PSUM") as lgp, \
             tc.tile_pool(name="lt_psum", bufs=2, space="PSUM") as ltp, \
             tc.tile_pool(name="lwork", bufs=1) as lwork:
            lps = lgp.tile([32, 1024], f32, tag="lps")
            for n in range(2):
                ns = slice(n * 512, (n + 1) * 512)
                for i in range(16):
                    nc.tensor.matmul(lps[:, ns], ct["w3s"][:, i, :],
                                     h2qs[i][:, ns], start=(i == 0), stop=False)
                nc.tensor.matmul(lps[:, ns], ct["embRs"][...],
                                 missT[0:64, ns], start=False, stop=False)
                nc.tensor.matmul(lps[:, ns], ct["embC8s"][64:72, :],
                                 missT[64:72, ns], start=False, stop=False)
                nc.tensor.matmul(lps[:, ns], ct["catlin"][64:96, :],
                                 xpT[64:96, ns], start=False, stop=True)
            lts = lwork.tile([32, 1024], f32r, tag="lts")
            nc.scalar.copy(out=lts[...], in_=lps[...])
            lout = lwork.tile([128, 8, 32], f32, tag="lout")
            for c in range(8):
                lt2 = ltp.tile([128, 32], f32r, tag="lt2")
                nc.tensor.transpose(lt2[...], lts[:, c * 128:(c + 1) * 128],
                                    ct["ident"][0:32, 0:32])
                nc.vector.tensor_add(lout[:, c, :], lt2[...], ct["biasB"][...])
            lo_view = log_d[...].rearrange("(c p) o -> p c o", p=128)
            nc.sync.dma_start(out=lo_view, in_=lout[...])

        # ---- Phase B: batch-major L3 + cat + output ----
        with tc.tile_pool(name="fb_psum", bufs=1, space="PSUM") as fbp, \
             tc.tile_pool(name="cat_psum", bufs=2, space="PSUM") as ctp, \
             tc.tile_pool(name="featw", bufs=2) as featw:
            for s in range(8):
                bs = slice(s * 128, (s + 1) * 128)
                fps = fbp.tile([128, 2048], f32, tag="fps")
                for t in range(16):
                    # one PSUM bank holds 4 quads (512 cols); only the first
                    # matmul touching a bank starts its accumulation group
                    nc.tensor.matmul(
                        fps[:, t * 128:(t + 1) * 128],
                        h2qs[t][:, bs], ct["l3w"][:, t, :],
                        start=(t % 4 == 0), stop=False,
                    )
                for g in range(4):
                    nc.tensor.matmul(
                        fps[:, g * 512:(g + 1) * 512],
                        missT[0:64, bs], ct["embR"][:, g * 512:(g + 1) * 512],
                        start=False, stop=True,
                    )
                cps = ctp.tile([128, 1024], f32, tag="cps")
                nc.tensor.matmul(cps[:, 0:512], xpT[64:96, bs],
                                 ct["catrhs"][64:96, 0:512], start=True, stop=False)
                nc.tensor.matmul(cps[:, 512:1024], xpT[64:96, bs],
                                 ct["catrhs"][64:96, 512:1024], start=True, stop=True)
                nc.tensor.matmul(cps[:, 0:256], missT[64:72, bs],
                                 ct["embC8"][64:72, :], start=False, stop=True)
                fc = featw.tile([128, 3072], f32, tag="featc")
                nc.scalar.copy(out=fc[:, 0:2048], in_=fps[...])
                nc.vector.tensor_copy(fc[:, 2048:3072], cps[...])
                nc.sync.dma_start(
                    out=feat_d[...].rearrange("b f o -> b (f o)")[bs, :],
                    in_=fc[...],
                )

    nc.compile()
    return nc


def _get_nc():
    global _BUILT
    if _BUILT is None:
        _BUILT = _build_nc()
    return _BUILT


def _run(inputs, trace=False):
    from concourse.bass_utils import run_bass_kernel_spmd

    tab = np.asarray(inputs["tabular"], np.float32)
    consts = _prep_consts(inputs)
    nc = _get_nc()
    in_maps = []
    for c in range(NCORES):
        m = {k: v for k, v in consts.items()}
        m["tab"] = np.ascontiguousarray(tab[c * BT:(c + 1) * BT])
        in_maps.append(m)
    bkr = run_bass_kernel_spmd(nc, in_maps, list(range(NCORES)), trace=trace)
    feats = np.concatenate([r["features_out"] for r in bkr.results], axis=0)
    logits = np.concatenate([r["logits_out"] for r in bkr.results], axis=0)
    return logits, feats, bkr


def kernel(**inputs):
    logits, feats, _ = _run(inputs, trace=False)
    return logits, feats


# revision 17
# speedup vs baseline: 1.8805x; 1.1755x over previous
"""Trainium2 Bass kernel for nn_BaseNAM (per-feature tiny MLPs / NAM).

Strategy
--------
Data-parallel over batch: 8 cores x 1024 rows each, no collectives.

Math trick for missing-value handling: with x' = x * (1 - miss),
    feat_masked = (1-m) * MLP(x) + m * emb
                = MLP(x') + m * (emb - MLP(0))
since for m in {0,1}: m=0 -> MLP(x); m=1 -> x'=0 -> MLP(0), and the
correction m*(emb - c) with c = MLP_f(0) (a host-precomputed constant)
fixes it up exactly.  This makes masking + missing-embedding injection
pure PSUM-accumulated matmuls -- no elementwise mask work on DVE/ACT.

Per-core pipeline (feature-major layout [feature-dims, batch] for the MLP):
  - PE-transpose tabular chunks -> xT/missT [96, 1024]; x'T = xT*(1-missT)
  - L0/L1/L2: per 2-feature pack, block-diagonal matmuls (K<=128), ACT/DVE
    evacuation with fused bias+relu -> h2 "quad" tiles [128=(4f x 32), 1024]
  - logits: stacked-W3 selector matmuls accumulate sum_f W3_f h2_f directly
    in PSUM, plus matmul corrections for miss-emb and cat features
  - L3 batch-major: psum[b, (f,o)] = h2quad_slice.T @ blockdiag(W3) (+ miss
    corrections, + cat features) -> features output is written batch-major,
    DMA'd contiguously.
"""

import sys

import numpy as np

sys.path.insert(0, "/opt/trn_rl_repo")

F_REAL = 64
F_CAT = 32
OUT = 32
B = 8192
NCORES = 8
BT = B // NCORES  # 1024 rows per core

_BUILT = None  # cached (nc, const_names)


def _wn(v, g):
    v = np.asarray(v, np.float32)
    g = np.asarray(g, np.float32)
    n = np.sqrt((v * v).sum(-1, keepdims=True)).astype(np.float32)
    return (g[..., None] * v / n).astype(np.float32)


def _prep_consts(inp):
    f32 = np.float32
    b0 = np.asarray(inp["b0"], f32)
    b1 = np.asarray(inp["b1"], f32)
    b2 = np.asarray(inp["b2"], f32)
    w0 = _wn(inp["v0"], inp["g0"])[:, :, 0]  # [64, 64] (f, j)
    W1 = _wn(inp["v1"], inp["g1"])           # [64, 64, 64] (f, o, i)
    W2 = _wn(inp["v2"], inp["g2"])           # [64, 32, 64]
    W3 = _wn(inp["v3"], inp["g3"])           # [64, 32, 32]
    CL = np.asarray(inp["cat_linear"], f32)  # [32, 32]
    ME = np.asarray(inp["missing_emb"], f32) # [24, 32] (cat 0:8, real 8:24)
    bias = np.asarray(inp["bias"], f32)      # [1, 32]

    # c_f = MLP_f(0)
    h0 = np.maximum(b0, 0.0)
    h1 = np.maximum(np.einsum("foi,fi->fo", W1, h0).astype(f32) + b1, 0.0)
    h2 = np.maximum(np.einsum("foi,fi->fo", W2, h1).astype(f32) + b2, 0.0)
    cf = np.einsum("foi,fi->fo", W3, h2).astype(f32)  # [64, 32]
    embp = (-cf).astype(f32)
    embp[:16] += ME[8:24]  # emb'_f = emb_f - c_f (f<16), -c_f otherwise

    C = {}
    # L0 selector weights: [k=64, pack=32, j=128]
    l0w = np.zeros((64, 32, 128), f32)
    for p in range(32):
        for s in range(2):
            l0w[2 * p + s, p, s * 64:(s + 1) * 64] = w0[2 * p + s]
    C["l0w"] = l0w
    # L1 block-diag: [k=128, pack, j=128]
    l1w = np.zeros((128, 32, 128), f32)
    for p in range(32):
        l1w[0:64, p, 0:64] = W1[2 * p].T
        l1w[64:128, p, 64:128] = W1[2 * p + 1].T
    C["l1w"] = l1w
    # L2 block-diag: [k=128, pack, j=64]
    import ml_dtypes
    bf16 = ml_dtypes.bfloat16
    l2w = np.zeros((128, 32, 64), f32)
    for p in range(32):
        l2w[0:64, p, 0:32] = W2[2 * p].T
        l2w[64:128, p, 32:64] = W2[2 * p + 1].T
    C["l2w"] = l2w.astype(bf16)
    # L3 block-diag per quad: [k=128, quad=16, j=128] (bf16: N=128 matmuls
    # run at full rate in bf16 but 4x slower in fp32/f32r)
    l3w = np.zeros((128, 16, 128), f32)
    for t in range(16):
        for q in range(4):
            l3w[q * 32:(q + 1) * 32, t, q * 32:(q + 1) * 32] = W3[4 * t + q].T
    C["l3w"] = l3w.astype(bf16)
    # stacked W3 for logits: [k=128, quad, o=32]
    w3s = np.zeros((128, 16, 32), f32)
    for t in range(16):
        for q in range(4):
            w3s[q * 32:(q + 1) * 32, t, :] = W3[4 * t + q].T
    C["w3s"] = w3s.astype(bf16)
    # biases, per-partition columns
    b0c = np.zeros((128, 32), f32)
    b1c = np.zeros((128, 32), f32)
    for p in range(32):
        b0c[0:64, p] = b0[2 * p]
        b0c[64:128, p] = b0[2 * p + 1]
        b1c[0:64, p] = b1[2 * p]
        b1c[64:128, p] = b1[2 * p + 1]
    C["b0c"] = b0c
    C["b1c"] = b1c
    b2q = np.zeros((128, 16), f32)
    for t in range(16):
        for q in range(4):
            b2q[q * 32:(q + 1) * 32, t] = b2[4 * t + q]
    C["b2q"] = b2q
    # miss-emb correction rhs, real features: [64, 2048]
    embR = np.zeros((64, 2048), f32)
    for f in range(64):
        embR[f, f * 32:(f + 1) * 32] = embp[f]
    C["embR"] = embR
    # cat-feature constants are padded so their data sits at partitions 64+,
    # matching the base partition of the xpT[64:96] / missT[64:72] operands
    # (matmul requires lhsT and rhs to share a base partition).
    # cat linear block-diag rhs: [96, 1024], rows 64:96
    catrhs = np.zeros((96, 1024), f32)
    for fc in range(32):
        catrhs[64 + fc, fc * 32:(fc + 1) * 32] = CL[fc]
    C["catrhs"] = catrhs
    # cat miss-emb rhs: [72, 256], rows 64:72
    embC8 = np.zeros((72, 256), f32)
    for fc in range(8):
        embC8[64 + fc, fc * 32:(fc + 1) * 32] = ME[fc]
    C["embC8"] = embC8
    # logits correction weights
    C["embRs"] = embp                      # [64, 32]
    embC8s = np.zeros((72, 32), f32)
    embC8s[64:72] = ME[:8]
    C["embC8s"] = embC8s
    catlin = np.zeros((96, 32), f32)
    catlin[64:96] = CL
    C["catlin"] = catlin
    C["biasB"] = np.broadcast_to(bias, (128, 32)).copy()  # [128, 32]
    C["ident"] = np.eye(128, dtype=f32)  # PE-transpose identity
    return C


# kind: f32r = matmul-only constant (float32r, full-rate PE at N>=256),
#        bf16 = L3/logits-path constant, f32 = ACT/DVE-read constant
_CONST_SPECS = [
    ("l0w", [64, 32, 128], "f32r"),
    ("l1w", [128, 32, 128], "f32r"),
    ("l2w", [128, 32, 64], "bf16"),
    ("l3w", [128, 16, 128], "bf16"),
    ("w3s", [128, 16, 32], "bf16"),
    ("b0c", [128, 32], "f32"),
    ("b1c", [128, 32], "f32"),
    ("b2q", [128, 16], "f32"),
    ("embR", [64, 2048], "f32r"),
    ("catrhs", [96, 1024], "f32r"),
    ("embC8", [72, 256], "f32r"),
    ("embRs", [64, 32], "f32r"),
    ("embC8s", [72, 32], "f32r"),
    ("catlin", [96, 32], "f32r"),
    ("biasB", [128, 32], "f32"),
    ("ident", [128, 128], "f32r"),
]


def _build_nc():
    import concourse.mybir as mybir
    import concourse.tile as tile
    from concourse import bacc
    from contextlib import ExitStack

    f32 = mybir.dt.float32
    f32r = mybir.dt.float32r
    bf16 = mybir.dt.bfloat16
    _DT = {"f32": f32, "f32r": f32r, "bf16": bf16}
    Relu = mybir.ActivationFunctionType.Relu
    mult = mybir.AluOpType.mult
    add = mybir.AluOpType.add
    amax = mybir.AluOpType.max

    nc = bacc.Bacc(None, target_bir_lowering=False)
    tab_d = nc.declare_dram_parameter("tab", [BT, 2, 96], f32r, isOutput=False)
    cdram = {}
    for name, shape, kind in _CONST_SPECS:
        cdram[name] = nc.declare_dram_parameter(name, shape, _DT[kind], isOutput=False)
    feat_d = nc.declare_dram_parameter("features_out", [BT, 96, 32], f32, isOutput=True)
    log_d = nc.declare_dram_parameter("logits_out", [BT, 32], f32, isOutput=True)

    with ExitStack() as ctx:
        tc = ctx.enter_context(tile.TileContext(nc))
        consts = ctx.enter_context(tc.tile_pool(name="consts", bufs=1))
        h2pool = ctx.enter_context(tc.tile_pool(name="h2", bufs=16))

        ct = {}
        for name, shape, kind in _CONST_SPECS:
            t = consts.tile(shape, _DT[kind], tag=name)
            nc.sync.dma_start(out=t[...], in_=cdram[name][...])
            ct[name] = t

        xpT = consts.tile([96, 1024], f32r, tag="xpT")    # x * (1-m), feature-major
        missT = consts.tile([96, 1024], f32r, tag="missT")

        # ---- Phase A0: transpose tabular to feature-major ----
        with tc.tile_pool(name="tp_psum", bufs=4, space="PSUM") as tpp, \
             tc.tile_pool(name="tabw", bufs=3) as tabw:
            for c in range(8):
                tabc = tabw.tile([128, 2, 96], f32r, tag="tabc")
                nc.sync.dma_start(
                    out=tabc[...],
                    in_=tab_d[c * 128:(c + 1) * 128, :, :],
                )
                pt = tpp.tile([96, 128], f32r, tag="pt")
                nc.tensor.transpose(pt[...], tabc[:, 0, :], ct["ident"][...])
                nc.scalar.copy(out=xpT[:, c * 128:(c + 1) * 128], in_=pt[...])
                pm = tpp.tile([96, 128], f32r, tag="pt")
                nc.tensor.transpose(pm[...], tabc[:, 1, :], ct["ident"][...])
                nc.scalar.copy(out=missT[:, c * 128:(c + 1) * 128], in_=pm[...])
            m1T = tabw.tile([96, 1024], f32, tag="m1T")
            # m1T = 1 - missT
            nc.vector.tensor_scalar(
                out=m1T[...], in0=missT[...], scalar1=-1.0, scalar2=1.0,
                op0=mult, op1=add,
            )
            # xpT *= m1T  (in-place: xpT currently holds raw x^T)
            nc.vector.tensor_mul(xpT[...], xpT[...], m1T[...])

        # ---- Phase A: per-pack MLP chain L0 -> L1 -> L2 ----
        # single-bank [128,512] psum tiles + per-chunk evacuation so the
        # Tile scheduler can pipeline across packs (keeps the PE stream
        # dense enough for the HAM clock to stay at 2.4 GHz)
        h2qs = []
        with tc.tile_pool(name="mm_psum", bufs=6, space="PSUM") as mmp, \
             tc.tile_pool(name="l2_psum", bufs=2, space="PSUM") as l2p, \
             tc.tile_pool(name="hwork", bufs=4) as hwork:
            for t in range(16):
                ps2 = [l2p.tile([128, 512], f32, tag="ps2", name=f"ps2_{t}_{_n}") for _n in range(2)]
                h2q = h2pool.tile([128, 1024], bf16, tag="h2q")
                for s2 in range(2):
                    p = 2 * t + s2
                    h0 = hwork.tile([128, 1024], f32r, tag="h0")
                    for n in range(2):
                        ps0 = mmp.tile([128, 512], f32, tag="ps")
                        nc.tensor.matmul(
                            ps0[...],
                            ct["l0w"][:, p, :],
                            xpT[0:64, n * 512:(n + 1) * 512],
                            start=True, stop=True,
                        )
                        nc.scalar.activation(h0[:, n * 512:(n + 1) * 512],
                                             ps0[...], Relu,
                                             bias=ct["b0c"][:, p:p + 1])
                    h1 = hwork.tile([128, 1024], bf16, tag="h1")
                    for n in range(2):
                        ps1 = mmp.tile([128, 512], f32, tag="ps")
                        nc.tensor.matmul(
                            ps1[...],
                            ct["l1w"][:, p, :],
                            h0[:, n * 512:(n + 1) * 512],
                            start=True, stop=True,
                        )
                        # relu(x + b) on DVE: max(x + b, 0)
                        nc.vector.tensor_scalar(
                            out=h1[:, n * 512:(n + 1) * 512], in0=ps1[...],
                            scalar1=ct["b1c"][:, p:p + 1], scalar2=0.0,
                            op0=add, op1=amax,
                        )
                    for n in range(2):
                        nc.tensor.matmul(
                            ps2[n][s2 * 64:(s2 + 1) * 64, :],
                            ct["l2w"][:, p, :],
                            h1[:, n * 512:(n + 1) * 512],
                            start=True, stop=True,
                            tile_position=(0, 64 * s2),
                        )
                for n in range(2):
                    nc.vector.tensor_scalar(
                        out=h2q[:, n * 512:(n + 1) * 512], in0=ps2[n][...],
                        scalar1=ct["b2q"][:, t:t + 1], scalar2=0.0,
                        op0=add, op1=amax,
                    )
                h2qs.append(h2q)

        # ---- Logits: accumulate sum_f W3_f h2_f (+ corrections) in PSUM ----
        with tc.tile_pool(name="lg_psum", bufs=1, space="PSUM") as lgp, \
             tc.tile_pool(name="lt_psum", bufs=2, space="PSUM") as ltp, \
             tc.tile_pool(name="lwork", bufs=1) as lwork:
            lps = lgp.tile([32, 1024], f32, tag="lps")
            for n in range(2):
                ns = slice(n * 512, (n + 1) * 512)
                for i in range(16):
                    nc.tensor.matmul(lps[:, ns], ct["w3s"][:, i, :],
                                     h2qs[i][:, ns], start=(i == 0), stop=False)
                nc.tensor.matmul(lps[:, ns], ct["embRs"][...],
                                 missT[0:64, ns], start=False, stop=False)
                nc.tensor.matmul(lps[:, ns], ct["embC8s"][64:72, :],
                                 missT[64:72, ns], start=False, stop=False)
                nc.tensor.matmul(lps[:, ns], ct["catlin"][64:96, :],
                                 xpT[64:96, ns], start=False, stop=True)
            lts = lwork.tile([32, 1024], f32r, tag="lts")
            nc.scalar.copy(out=lts[...], in_=lps[...])
            lout = lwork.tile([128, 8, 32], f32, tag="lout")
            for c in range(8):
                lt2 = ltp.tile([128, 32], f32r, tag="lt2")
                nc.tensor.transpose(lt2[...], lts[:, c * 128:(c + 1) * 128],
                                    ct["ident"][0:32, 0:32])
                nc.vector.tensor_add(lout[:, c, :], lt2[...], ct["biasB"][...])
            lo_view = log_d[...].rearrange("(c p) o -> p c o", p=128)
            nc.sync.dma_start(out=lo_view, in_=lout[...])

        # ---- Phase B: batch-major L3 + cat + output ----
        with tc.tile_pool(name="fb_psum", bufs=3, space="PSUM") as fbp, \
             tc.tile_pool(name="cat_psum", bufs=1, space="PSUM") as ctp, \
             tc.tile_pool(name="featw", bufs=2) as featw:
            for s in range(8):
                bs = slice(s * 128, (s + 1) * 128)
                fps = [fbp.tile([128, 1024], f32, tag="fps", name=f"fps_{s}_{_n}") for _n in range(2)]
                for t in range(16):
                    # one PSUM bank holds 4 quads (512 cols); only the first
                    # matmul touching a bank starts its accumulation group
                    nc.tensor.matmul(
                        fps[t // 8][:, (t % 8) * 128:(t % 8 + 1) * 128],
                        h2qs[t][:, bs], ct["l3w"][:, t, :],
                        start=(t % 4 == 0), stop=False,
                    )
                for g in range(4):
                    nc.tensor.matmul(
                        fps[g // 2][:, (g % 2) * 512:(g % 2 + 1) * 512],
                        missT[0:64, bs], ct["embR"][:, g * 512:(g + 1) * 512],
                        start=False, stop=True,
                    )
                cps = ctp.tile([128, 1024], f32, tag="cps")
                nc.tensor.matmul(cps[:, 0:512], xpT[64:96, bs],
                                 ct["catrhs"][64:96, 0:512], start=True, stop=False)
                nc.tensor.matmul(cps[:, 512:1024], xpT[64:96, bs],
                                 ct["catrhs"][64:96, 512:1024], start=True, stop=True)
                nc.tensor.matmul(cps[:, 0:256], missT[64:72, bs],
                                 ct["embC8"][64:72, :], start=False, stop=True)
                fc = featw.tile([128, 3072], f32, tag="featc")
                nc.scalar.copy(out=fc[:, 0:1024], in_=fps[0][...])
                nc.scalar.copy(out=fc[:, 1024:2048], in_=fps[1][...])
                nc.vector.tensor_copy(fc[:, 2048:3072], cps[...])
                nc.sync.dma_start(
                    out=feat_d[...].rearrange("b f o -> b (f o)")[bs, :],
                    in_=fc[...],
                )

    nc.compile()
    return nc


def _get_nc():
    global _BUILT
    if _BUILT is None:
        _BUILT = _build_nc()
    return _BUILT


def _run(inputs, trace=False):
    from concourse.bass_utils import run_bass_kernel_spmd

    tab = np.asarray(inputs["tabular"], np.float32)
    consts = _prep_consts(inputs)
    nc = _get_nc()
    in_maps = []
    for c in range(NCORES):
        m = {k: v for k, v in consts.items()}
        m["tab"] = np.ascontiguousarray(tab[c * BT:(c + 1) * BT])
        in_maps.append(m)
    bkr = run_bass_kernel_spmd(nc, in_maps, list(range(NCORES)), trace=trace)
    feats = np.concatenate([r["features_out"] for r in bkr.results], axis=0)
    logits = np.concatenate([r["logits_out"] for r in bkr.results], axis=0)
    return logits, feats, bkr


def kernel(**inputs):
    logits, feats, _ = _run(inputs, trace=False)
    return logits, feats


# revision 19
# speedup vs baseline: 2.4622x; 1.3093x over previous
"""Trainium2 Bass kernel for nn_BaseNAM (per-feature tiny MLPs / NAM).

Strategy
--------
Data-parallel over batch: 8 cores x 1024 rows each, no collectives.

Math trick for missing-value handling: with x' = x * (1 - miss),
    feat_masked = (1-m) * MLP(x) + m * emb
                = MLP(x') + m * (emb - MLP(0))
since for m in {0,1}: m=0 -> MLP(x); m=1 -> x'=0 -> MLP(0), and the
correction m*(emb - c) with c = MLP_f(0) (a host-precomputed constant)
fixes it up exactly.  This makes masking + missing-embedding injection
pure PSUM-accumulated matmuls -- no elementwise mask work on DVE/ACT.

Per-core pipeline (feature-major layout [feature-dims, batch] for the MLP):
  - PE-transpose tabular chunks -> xT/missT [96, 1024]; x'T = xT*(1-missT)
  - L0/L1/L2: per 2-feature pack, block-diagonal matmuls (K<=128), ACT/DVE
    evacuation with fused bias+relu -> h2 "quad" tiles [128=(4f x 32), 1024]
  - logits: stacked-W3 selector matmuls accumulate sum_f W3_f h2_f directly
    in PSUM, plus matmul corrections for miss-emb and cat features
  - L3 batch-major: psum[b, (f,o)] = h2quad_slice.T @ blockdiag(W3) (+ miss
    corrections, + cat features) -> features output is written batch-major,
    DMA'd contiguously.
"""

import sys

import numpy as np

sys.path.insert(0, "/opt/trn_rl_repo")

F_REAL = 64
F_CAT = 32
OUT = 32
B = 8192
NCORES = 8
BT = B // NCORES  # 1024 rows per core

_BUILT = None  # cached (nc, const_names)


def _wn(v, g):
    v = np.asarray(v, np.float32)
    g = np.asarray(g, np.float32)
    n = np.sqrt((v * v).sum(-1, keepdims=True)).astype(np.float32)
    return (g[..., None] * v / n).astype(np.float32)


def _prep_consts(inp):
    f32 = np.float32
    b0 = np.asarray(inp["b0"], f32)
    b1 = np.asarray(inp["b1"], f32)
    b2 = np.asarray(inp["b2"], f32)
    w0 = _wn(inp["v0"], inp["g0"])[:, :, 0]  # [64, 64] (f, j)
    W1 = _wn(inp["v1"], inp["g1"])           # [64, 64, 64] (f, o, i)
    W2 = _wn(inp["v2"], inp["g2"])           # [64, 32, 64]
    W3 = _wn(inp["v3"], inp["g3"])           # [64, 32, 32]
    CL = np.asarray(inp["cat_linear"], f32)  # [32, 32]
    ME = np.asarray(inp["missing_emb"], f32) # [24, 32] (cat 0:8, real 8:24)
    bias = np.asarray(inp["bias"], f32)      # [1, 32]

    # c_f = MLP_f(0)
    h0 = np.maximum(b0, 0.0)
    h1 = np.maximum(np.einsum("foi,fi->fo", W1, h0).astype(f32) + b1, 0.0)
    h2 = np.maximum(np.einsum("foi,fi->fo", W2, h1).astype(f32) + b2, 0.0)
    cf = np.einsum("foi,fi->fo", W3, h2).astype(f32)  # [64, 32]
    embp = (-cf).astype(f32)
    embp[:16] += ME[8:24]  # emb'_f = emb_f - c_f (f<16), -c_f otherwise

    fp16 = np.float16
    C = {}
    # L0 selector weights: [k=64, pack=32, j=128]
    l0w = np.zeros((64, 32, 128), f32)
    for p in range(32):
        for s in range(2):
            l0w[2 * p + s, p, s * 64:(s + 1) * 64] = w0[2 * p + s]
    C["l0w"] = l0w.astype(fp16)
    # L1 block-diag: [k=128, pack, j=128]
    l1w = np.zeros((128, 32, 128), f32)
    for p in range(32):
        l1w[0:64, p, 0:64] = W1[2 * p].T
        l1w[64:128, p, 64:128] = W1[2 * p + 1].T
    C["l1w"] = l1w.astype(fp16)
    # L2 block-diag: [k=128, pack, j=64]
    fp16 = np.float16
    l2w = np.zeros((128, 32, 64), f32)
    for p in range(32):
        l2w[0:64, p, 0:32] = W2[2 * p].T
        l2w[64:128, p, 32:64] = W2[2 * p + 1].T
    C["l2w"] = l2w.astype(fp16)
    # L3 block-diag per quad: [k=128, quad=16, j=128] (bf16: N=128 matmuls
    # run at full rate in bf16 but 4x slower in fp32/f32r)
    l3w = np.zeros((128, 16, 128), f32)
    for t in range(16):
        for q in range(4):
            l3w[q * 32:(q + 1) * 32, t, q * 32:(q + 1) * 32] = W3[4 * t + q].T
    C["l3w"] = l3w.astype(fp16)
    # stacked W3 for logits: [k=128, quad, o=32]
    w3s = np.zeros((128, 16, 32), f32)
    for t in range(16):
        for q in range(4):
            w3s[q * 32:(q + 1) * 32, t, :] = W3[4 * t + q].T
    C["w3s"] = w3s.astype(fp16)
    # biases, per-partition columns
    b0c = np.zeros((128, 32), f32)
    b1c = np.zeros((128, 32), f32)
    for p in range(32):
        b0c[0:64, p] = b0[2 * p]
        b0c[64:128, p] = b0[2 * p + 1]
        b1c[0:64, p] = b1[2 * p]
        b1c[64:128, p] = b1[2 * p + 1]
    C["b0c"] = b0c
    C["b1c"] = b1c
    b2q = np.zeros((128, 16), f32)
    for t in range(16):
        for q in range(4):
            b2q[q * 32:(q + 1) * 32, t] = b2[4 * t + q]
    C["b2q"] = b2q
    # miss-emb correction rhs, real features: [64, 2048]
    embR = np.zeros((64, 2048), f32)
    for f in range(64):
        embR[f, f * 32:(f + 1) * 32] = embp[f]
    C["embR"] = embR.astype(fp16)
    # cat-feature constants are padded so their data sits at partitions 64+,
    # matching the base partition of the xpT[64:96] / missT[64:72] operands
    # (matmul requires lhsT and rhs to share a base partition).
    # cat linear block-diag rhs: [96, 1024], rows 64:96
    catrhs = np.zeros((96, 1024), f32)
    for fc in range(32):
        catrhs[64 + fc, fc * 32:(fc + 1) * 32] = CL[fc]
    C["catrhs"] = catrhs.astype(fp16)
    # cat miss-emb rhs: [72, 256], rows 64:72
    embC8 = np.zeros((72, 256), f32)
    for fc in range(8):
        embC8[64 + fc, fc * 32:(fc + 1) * 32] = ME[fc]
    C["embC8"] = embC8.astype(fp16)
    # logits correction weights
    C["embRs"] = embp.astype(fp16)             # [64, 32]
    embC8s = np.zeros((72, 32), f32)
    embC8s[64:72] = ME[:8]
    C["embC8s"] = embC8s.astype(fp16)
    catlin = np.zeros((96, 32), f32)
    catlin[64:96] = CL
    C["catlin"] = catlin.astype(fp16)
    C["biasB"] = np.broadcast_to(bias, (128, 32)).copy()  # [128, 32]
    C["ident"] = np.eye(128, dtype=f32)  # PE-transpose identity
    return C


# kind: f32r = matmul-only constant (float32r, full-rate PE at N>=256),
#        bf16 = L3/logits-path constant, f32 = ACT/DVE-read constant
_CONST_SPECS = [
    ("l0w", [64, 32, 128], "fp16"),
    ("l1w", [128, 32, 128], "fp16"),
    ("l2w", [128, 32, 64], "fp16"),
    ("l3w", [128, 16, 128], "fp16"),
    ("w3s", [128, 16, 32], "fp16"),
    ("b0c", [128, 32], "f32"),
    ("b1c", [128, 32], "f32"),
    ("b2q", [128, 16], "f32"),
    ("embR", [64, 2048], "fp16"),
    ("catrhs", [96, 1024], "fp16"),
    ("embC8", [72, 256], "fp16"),
    ("embRs", [64, 32], "fp16"),
    ("embC8s", [72, 32], "fp16"),
    ("catlin", [96, 32], "fp16"),
    ("biasB", [128, 32], "f32"),
    ("ident", [128, 128], "f32r"),
]


def _build_nc():
    import concourse.mybir as mybir
    import concourse.tile as tile
    from concourse import bacc
    from contextlib import ExitStack

    f32 = mybir.dt.float32
    f32r = mybir.dt.float32r
    bf16 = mybir.dt.bfloat16
    fp16 = mybir.dt.float16
    _DT = {"f32": f32, "f32r": f32r, "bf16": bf16, "fp16": fp16}
    Relu = mybir.ActivationFunctionType.Relu
    mult = mybir.AluOpType.mult
    add = mybir.AluOpType.add
    amax = mybir.AluOpType.max

    nc = bacc.Bacc(None, target_bir_lowering=False)
    tab_d = nc.declare_dram_parameter("tab", [BT, 2, 96], f32r, isOutput=False)
    cdram = {}
    for name, shape, kind in _CONST_SPECS:
        cdram[name] = nc.declare_dram_parameter(name, shape, _DT[kind], isOutput=False)
    feat_d = nc.declare_dram_parameter("features_out", [BT, 96, 32], f32, isOutput=True)
    log_d = nc.declare_dram_parameter("logits_out", [BT, 32], f32, isOutput=True)

    with ExitStack() as ctx:
        tc = ctx.enter_context(tile.TileContext(nc))
        consts = ctx.enter_context(tc.tile_pool(name="consts", bufs=1))
        h2pool = ctx.enter_context(tc.tile_pool(name="h2", bufs=16))

        ct = {}
        for name, shape, kind in _CONST_SPECS:
            t = consts.tile(shape, _DT[kind], tag=name)
            nc.sync.dma_start(out=t[...], in_=cdram[name][...])
            ct[name] = t

        xpT = consts.tile([96, 1024], fp16, tag="xpT")    # x * (1-m), feature-major
        missT = consts.tile([96, 1024], fp16, tag="missT")

        # ---- Phase A0: transpose tabular to feature-major ----
        with tc.tile_pool(name="tp_psum", bufs=4, space="PSUM") as tpp, \
             tc.tile_pool(name="tabw", bufs=3) as tabw:
            for c in range(8):
                tabc = tabw.tile([128, 2, 96], f32r, tag="tabc")
                nc.sync.dma_start(
                    out=tabc[...],
                    in_=tab_d[c * 128:(c + 1) * 128, :, :],
                )
                pt = tpp.tile([96, 128], f32r, tag="pt")
                nc.tensor.transpose(pt[...], tabc[:, 0, :], ct["ident"][...])
                nc.scalar.copy(out=xpT[:, c * 128:(c + 1) * 128], in_=pt[...])
                pm = tpp.tile([96, 128], f32r, tag="pt")
                nc.tensor.transpose(pm[...], tabc[:, 1, :], ct["ident"][...])
                nc.scalar.copy(out=missT[:, c * 128:(c + 1) * 128], in_=pm[...])
            m1T = tabw.tile([96, 1024], fp16, tag="m1T")
            # m1T = 1 - missT
            nc.vector.tensor_scalar(
                out=m1T[...], in0=missT[...], scalar1=-1.0, scalar2=1.0,
                op0=mult, op1=add,
            )
            # xpT *= m1T  (in-place: xpT currently holds raw x^T)
            nc.vector.tensor_mul(xpT[...], xpT[...], m1T[...])

        # ---- Phase A: per-pack MLP chain L0 -> L1 -> L2 ----
        # single-bank [128,512] psum tiles + per-chunk evacuation so the
        # Tile scheduler can pipeline across packs (keeps the PE stream
        # dense enough for the HAM clock to stay at 2.4 GHz)
        h2qs = []
        with tc.tile_pool(name="mm_psum", bufs=6, space="PSUM") as mmp, \
             tc.tile_pool(name="l2_psum", bufs=2, space="PSUM") as l2p, \
             tc.tile_pool(name="hwork", bufs=4) as hwork:
            for t in range(16):
                ps2 = [l2p.tile([128, 512], f32, tag="ps2", name=f"ps2_{t}_{_n}") for _n in range(2)]
                h2q = h2pool.tile([128, 1024], fp16, tag="h2q")
                for s2 in range(2):
                    p = 2 * t + s2
                    h0 = hwork.tile([128, 1024], fp16, tag="h0")
                    for n in range(2):
                        ps0 = mmp.tile([128, 512], f32, tag="ps")
                        nc.tensor.matmul(
                            ps0[...],
                            ct["l0w"][:, p, :],
                            xpT[0:64, n * 512:(n + 1) * 512],
                            start=True, stop=True,
                        )
                        nc.scalar.activation(h0[:, n * 512:(n + 1) * 512],
                                             ps0[...], Relu,
                                             bias=ct["b0c"][:, p:p + 1])
                    h1 = hwork.tile([128, 1024], fp16, tag="h1")
                    for n in range(2):
                        ps1 = mmp.tile([128, 512], f32, tag="ps")
                        nc.tensor.matmul(
                            ps1[...],
                            ct["l1w"][:, p, :],
                            h0[:, n * 512:(n + 1) * 512],
                            start=True, stop=True,
                        )
                        # relu(x + b) on DVE: max(x + b, 0)
                        nc.vector.tensor_scalar(
                            out=h1[:, n * 512:(n + 1) * 512], in0=ps1[...],
                            scalar1=ct["b1c"][:, p:p + 1], scalar2=0.0,
                            op0=add, op1=amax,
                        )
                    for n in range(2):
                        nc.tensor.matmul(
                            ps2[n][s2 * 64:(s2 + 1) * 64, :],
                            ct["l2w"][:, p, :],
                            h1[:, n * 512:(n + 1) * 512],
                            start=True, stop=True,
                            tile_position=(0, 64 * s2),
                        )
                for n in range(2):
                    nc.vector.tensor_scalar(
                        out=h2q[:, n * 512:(n + 1) * 512], in0=ps2[n][...],
                        scalar1=ct["b2q"][:, t:t + 1], scalar2=0.0,
                        op0=add, op1=amax,
                    )
                h2qs.append(h2q)

        # ---- Logits: accumulate sum_f W3_f h2_f (+ corrections) in PSUM ----
        with tc.tile_pool(name="lg_psum", bufs=1, space="PSUM") as lgp, \
             tc.tile_pool(name="lt_psum", bufs=2, space="PSUM") as ltp, \
             tc.tile_pool(name="lwork", bufs=1) as lwork:
            lps = lgp.tile([32, 1024], f32, tag="lps")
            for n in range(2):
                ns = slice(n * 512, (n + 1) * 512)
                for i in range(16):
                    nc.tensor.matmul(lps[:, ns], ct["w3s"][:, i, :],
                                     h2qs[i][:, ns], start=(i == 0), stop=False)
                nc.tensor.matmul(lps[:, ns], ct["embRs"][...],
                                 missT[0:64, ns], start=False, stop=False)
                nc.tensor.matmul(lps[:, ns], ct["embC8s"][64:72, :],
                                 missT[64:72, ns], start=False, stop=False)
                nc.tensor.matmul(lps[:, ns], ct["catlin"][64:96, :],
                                 xpT[64:96, ns], start=False, stop=True)
            lts = lwork.tile([32, 1024], f32r, tag="lts")
            nc.scalar.copy(out=lts[...], in_=lps[...])
            lout = lwork.tile([128, 8, 32], f32, tag="lout")
            for c in range(8):
                lt2 = ltp.tile([128, 32], f32r, tag="lt2")
                nc.tensor.transpose(lt2[...], lts[:, c * 128:(c + 1) * 128],
                                    ct["ident"][0:32, 0:32])
                nc.vector.tensor_add(lout[:, c, :], lt2[...], ct["biasB"][...])
            lo_view = log_d[...].rearrange("(c p) o -> p c o", p=128)
            nc.sync.dma_start(out=lo_view, in_=lout[...])

        # ---- Phase B: batch-major L3 + cat + output ----
        with tc.tile_pool(name="fb_psum", bufs=3, space="PSUM") as fbp, \
             tc.tile_pool(name="cat_psum", bufs=1, space="PSUM") as ctp, \
             tc.tile_pool(name="featw", bufs=2) as featw:
            for s in range(8):
                bs = slice(s * 128, (s + 1) * 128)
                fps = [fbp.tile([128, 1024], f32, tag="fps", name=f"fps_{s}_{_n}") for _n in range(2)]
                for t in range(16):
                    # one PSUM bank holds 4 quads (512 cols); only the first
                    # matmul touching a bank starts its accumulation group
                    nc.tensor.matmul(
                        fps[t // 8][:, (t % 8) * 128:(t % 8 + 1) * 128],
                        h2qs[t][:, bs], ct["l3w"][:, t, :],
                        start=(t % 4 == 0), stop=False,
                    )
                for g in range(4):
                    nc.tensor.matmul(
                        fps[g // 2][:, (g % 2) * 512:(g % 2 + 1) * 512],
                        missT[0:64, bs], ct["embR"][:, g * 512:(g + 1) * 512],
                        start=False, stop=True,
                    )
                cps = ctp.tile([128, 1024], f32, tag="cps")
                nc.tensor.matmul(cps[:, 0:512], xpT[64:96, bs],
                                 ct["catrhs"][64:96, 0:512], start=True, stop=False)
                nc.tensor.matmul(cps[:, 512:1024], xpT[64:96, bs],
                                 ct["catrhs"][64:96, 512:1024], start=True, stop=True)
                nc.tensor.matmul(cps[:, 0:256], missT[64:72, bs],
                                 ct["embC8"][64:72, :], start=False, stop=True)
                fc = featw.tile([128, 3072], f32, tag="featc")
                nc.scalar.copy(out=fc[:, 0:1024], in_=fps[0][...])
                nc.scalar.copy(out=fc[:, 1024:2048], in_=fps[1][...])
                nc.vector.tensor_copy(fc[:, 2048:3072], cps[...])
                nc.sync.dma_start(
                    out=feat_d[...].rearrange("b f o -> b (f o)")[bs, :],
                    in_=fc[...],
                )

    nc.compile()
    return nc


def _get_nc():
    global _BUILT
    if _BUILT is None:
        _BUILT = _build_nc()
    return _BUILT


def _run(inputs, trace=False):
    from concourse.bass_utils import run_bass_kernel_spmd

    tab = np.asarray(inputs["tabular"], np.float32)
    consts = _prep_consts(inputs)
    nc = _get_nc()
    in_maps = []
    for c in range(NCORES):
        m = {k: v for k, v in consts.items()}
        m["tab"] = np.ascontiguousarray(tab[c * BT:(c + 1) * BT])
        in_maps.append(m)
    bkr = run_bass_kernel_spmd(nc, in_maps, list(range(NCORES)), trace=trace)
    feats = np.concatenate([r["features_out"] for r in bkr.results], axis=0)
    logits = np.concatenate([r["logits_out"] for r in bkr.results], axis=0)
    return logits, feats, bkr


def kernel(**inputs):
    logits, feats, _ = _run(inputs, trace=False)
    return logits, feats
